# revision 24
# baseline (speedup 1.0000x reference)
"""Trainium2 Bass kernel for AttnBlock (GroupNorm + single-head spatial
self-attention + projection + residual).

Sharding: 8 cores = 4 batches x 2 query-halves. Each core computes
GN-folded K / VP for its batch (duplicated within the pair) and
attention for its half of the 4096 query positions. No collectives.

Math per core (batch b, query half h, N=4096 keys, NQ=2048 queries):
  GN:  h = a*x + c2 per channel (a = gns/std, c2 = gnb - mean*a)
  Fold GN into weights: W' = diag(a) W, bias chains use c2p = c2/a.
  K   = Wk'^T x                      [C, N]   (K-side affine offset is
                                     constant per query -> softmax-invariant,
                                     dropped entirely)
  Q   = Wq'^T x_q + (Wq'^T c2p + bq) [C, NQ]
  VPT = x^T Wvp' (+ c2p^T Wvp' folded into bpp)  with Wvp = (Wp Wv)^T
        so attention output is already projected.
  S^T = K^T Q * C^-0.5 -> E = exp(S^T)   [N, NQ] (no max-sub; scores O(5))
  O   = VPT^T @ E (unnormalized)     [C, NQ];  den = sum_j E
  out = x_q + O/den + bpp_dyn        bpp_dyn = Wp@bv + bp + Wvp'^T c2p
All matmuls in bf16 (FWL weight loads), accumulation fp32 in PSUM.
"""
import math
import numpy as np

import concourse.bass as bass
import concourse.bacc as bacc
import concourse.tile as tile
from concourse import mybir
from concourse.bass_utils import run_bass_kernel_spmd

F32 = mybir.dt.float32
BF16 = mybir.dt.bfloat16
AF = mybir.ActivationFunctionType
ALU = mybir.AluOpType

C = 512          # channels
N = 4096         # spatial positions (keys)
NQ = 2048        # queries per core
CT = 4           # channel tiles of 128
ICN = 4          # query chunks per core
ICW = 512        # query chunk width
JBN = 32         # j-blocks (128 wide)
GROUPS = 32
EPS = 1e-6
INV = 1.0 / math.sqrt(C)
BN_FMAX = 512


def _copy(eng, nc, out, in_):
    if eng is nc.scalar:
        nc.scalar.copy(out=out, in_=in_)
    else:
        eng.tensor_copy(out=out, in_=in_)


def _emit(nc, tc, ctx, tens, rep):
    r = f"r{rep}_"
    XF = tens["XF"]
    WQT, WKT, WVPT = tens["WQT"], tens["WKT"], tens["WVPT"]
    GM, OUT = tens["GM"], tens["OUT"]

    const = ctx.enter_context(tc.tile_pool(name=r + "const", bufs=1))
    kqpool = ctx.enter_context(tc.tile_pool(name=r + "kq", bufs=1))
    vpool = ctx.enter_context(tc.tile_pool(name=r + "vt", bufs=1))
    pps = ctx.enter_context(tc.tile_pool(name=r + "pps", bufs=4, space="PSUM"))

    # ---------------- DMA in: x, constants, weights ----------------
    # SBUF pool stack is LIFO: create in reverse order of release.
    xbpool = tc.alloc_tile_pool(name=r + "xb", bufs=1)
    wb = tc.alloc_tile_pool(name=r + "wb", bufs=1)
    pgn = tc.alloc_tile_pool(name=r + "pgn", bufs=2)
    wstage = tc.alloc_tile_pool(name=r + "wstage", bufs=1)
    xpool = tc.alloc_tile_pool(name=r + "xp", bufs=6)
    gps = tc.alloc_tile_pool(name=r + "gps", bufs=2, space="PSUM")

    gm_t = const.tile([128, 128], F32, name=r + "gm")
    nc.sync.dma_start(out=gm_t, in_=GM[:, :])
    cvec = const.tile([128, 16], F32, name=r + "cvec")
    nc.sync.dma_start(out=cvec, in_=tens["CVEC"][:, :])
    bq_t = [cvec[:, cb:cb + 1] for cb in range(CT)]
    bpp_t = [cvec[:, 4 + cb:5 + cb] for cb in range(CT)]
    gns_t = [cvec[:, 8 + t:9 + t] for t in range(CT)]
    gnb_t = [cvec[:, 12 + t:13 + t] for t in range(CT)]
    eps_t = const.tile([128, 1], F32, name=r + "eps")
    nc.vector.memset(eps_t, EPS)
    ones_t = const.tile([128, 128], F32, name=r + "ones")
    nc.vector.memset(ones_t, 1.0)
    a_t = [const.tile([128, 1], F32, name=f"{r}a{t}", tag=f"a{t}") for t in range(CT)]
    c2p_t = [const.tile([128, 1], BF16, name=f"{r}c2p{t}", tag=f"c2p{t}")
             for t in range(CT)]
    bqd_t = [const.tile([128, 1], F32, name=f"{r}bqd{cb}", tag=f"bqd{cb}")
             for cb in range(CT)]
    bppd_t = [const.tile([128, 1], F32, name=f"{r}bppd{cb}", tag=f"bppd{cb}")
              for cb in range(CT)]

    # Host rotates each core's XF so its 2048 query columns are always
    # columns 0:NQ (attention is permutation-invariant over keys).

    # ============ PHASE 1: stats, folds, K/Q/VPT ============
    if True:
        wq_f = [wstage.tile([128, C], F32, name=f"{r}wqf{t}", tag=f"wqf{t}")
                for t in range(CT)]
        wk_f = [wstage.tile([128, C], F32, name=f"{r}wkf{t}", tag=f"wkf{t}")
                for t in range(CT)]
        wvp_f = [wstage.tile([128, C], F32, name=f"{r}wvpf{t}", tag=f"wvpf{t}")
                 for t in range(CT)]
        for t in range(CT):
            nc.sync.dma_start(out=wk_f[t], in_=WKT[t])
        for t in range(CT):
            nc.sync.dma_start(out=wq_f[t], in_=WQT[t])
        for t in range(CT):
            nc.sync.dma_start(out=wvp_f[t], in_=WVPT[t])

        # --- stream x in: DMA chunk -> bn_stats(f32) + bf16 convert ---
        stats = [pgn.tile([128, N // BN_FMAX, 6], F32, name=f"{r}st{t}",
                          tag=f"st{t}") for t in range(CT)]
        xb_t = [xbpool.tile([128, N], BF16, name=f"{r}xb{t}", tag=f"xb{t}")
                for t in range(CT)]
        for t in range(CT):
            for s in range(N // BN_FMAX):
                sl = slice(s * BN_FMAX, (s + 1) * BN_FMAX)
                xs = xpool.tile([128, BN_FMAX], F32, name=f"{r}xs{t}_{s}",
                                tag="xs")
                nc.sync.dma_start(out=xs,
                                  in_=XF[t * 128:(t + 1) * 128, sl])
                nc.vector.bn_stats(out=stats[t][:, s, :], in_=xs)
                _copy([nc.scalar, nc.gpsimd][s % 2], nc, xb_t[t][:, sl], xs)

        # --- group stats -> a, c2p ---
        for t in range(CT):
            mv = pgn.tile([128, 2], F32, name=f"{r}mv{t}", tag="mv")
            nc.vector.bn_aggr(out=mv, in_=stats[t])
            t2 = pgn.tile([128, 2], F32, name=f"{r}t2{t}", tag="t2")
            nc.vector.tensor_copy(out=t2[:, 0:1], in_=mv[:, 0:1])
            sq = pgn.tile([128, 1], F32, name=f"{r}sq{t}", tag="sq")
            nc.vector.tensor_mul(out=sq, in0=mv[:, 0:1], in1=mv[:, 0:1])
            nc.vector.tensor_add(out=t2[:, 1:2], in0=mv[:, 1:2], in1=sq)
            chp = gps.tile([128, 2], F32, name=f"{r}chp{t}", tag="gp")
            nc.tensor.matmul(chp, gm_t, t2, start=True, stop=True)
            ch = pgn.tile([128, 2], F32, name=f"{r}ch{t}", tag="ch")
            nc.vector.tensor_copy(out=ch, in_=chp)
            gmean, gmsq = ch[:, 0:1], ch[:, 1:2]
            sg = pgn.tile([128, 1], F32, name=f"{r}sg{t}", tag="sg")
            nc.vector.tensor_mul(out=sg, in0=gmean, in1=gmean)
            gv = pgn.tile([128, 1], F32, name=f"{r}gv{t}", tag="gv")
            nc.vector.tensor_sub(out=gv, in0=gmsq, in1=sg)
            nc.scalar.activation(out=gv, in_=gv, func=AF.Sqrt, bias=eps_t, scale=1.0)
            nc.vector.reciprocal(out=gv, in_=gv)
            nc.vector.tensor_mul(out=a_t[t], in0=gv, in1=gns_t[t])
            # c2p = c2/a = gnb/a - gmean
            ia = pgn.tile([128, 1], F32, name=f"{r}ia{t}", tag="ia")
            nc.vector.reciprocal(out=ia, in_=a_t[t])
            c2p_f = pgn.tile([128, 1], F32, name=f"{r}c2f{t}", tag="c2f")
            nc.vector.tensor_mul(out=c2p_f, in0=gnb_t[t], in1=ia)
            nc.vector.tensor_sub(out=c2p_f, in0=c2p_f, in1=gmean)
            nc.vector.tensor_copy(out=c2p_t[t], in_=c2p_f)

        xpool.release()  # raw f32 x no longer needed (stats + bf16 copy done)

        # resident K, Q, VPT (bf16) — created after x release for SBUF room
        k_sb = [kqpool.tile([128, N], BF16, name=f"{r}k{t}", tag=f"k{t}")
                for t in range(CT)]
        q_sb = [kqpool.tile([128, NQ], BF16, name=f"{r}q{t}", tag=f"q{t}")
                for t in range(CT)]
        vt_sb = [vpool.tile([128, 512], BF16, name=f"{r}vt{jb}", tag=f"vt{jb}")
                 for jb in range(JBN)]

        # --- scaled bf16 weights: w' = a * w ---
        wqb = [wb.tile([128, C], BF16, name=f"{r}wqb{t}", tag=f"wqb{t}")
               for t in range(CT)]
        wkb = [wb.tile([128, C], BF16, name=f"{r}wkb{t}", tag=f"wkb{t}")
               for t in range(CT)]
        wvpb = [wb.tile([128, C], BF16, name=f"{r}wvpb{t}", tag=f"wvpb{t}")
                for t in range(CT)]
        for t in range(CT):
            nc.vector.tensor_scalar(out=wkb[t], in0=wk_f[t], scalar1=a_t[t],
                                    scalar2=None, op0=ALU.mult)
            nc.gpsimd.tensor_scalar(out=wqb[t], in0=wq_f[t], scalar1=a_t[t],
                                    scalar2=None, op0=ALU.mult)
            nc.scalar.activation(out=wvpb[t], in_=wvp_f[t], func=AF.Identity,
                                 scale=a_t[t])
        wstage.release()
        pgn.release()

        # --- dynamic biases: bq_dyn = bq + Wq'^T c2p ; bpp_dyn = bpp + Wvp'^T c2p
        for cb in range(CT):
            bp1 = gps.tile([128, 1], F32, name=f"{r}bp1{cb}", tag="gp")
            for t in range(CT):
                nc.tensor.matmul(bp1, wqb[t][:, cb * 128:(cb + 1) * 128],
                                 c2p_t[t], start=(t == 0), stop=(t == CT - 1))
            nc.vector.tensor_add(out=bqd_t[cb], in0=bp1, in1=bq_t[cb])
            bp2 = gps.tile([128, 1], F32, name=f"{r}bp2{cb}", tag="gp")
            for t in range(CT):
                nc.tensor.matmul(bp2, wvpb[t][:, cb * 128:(cb + 1) * 128],
                                 c2p_t[t], start=(t == 0), stop=(t == CT - 1))
            nc.vector.tensor_add(out=bppd_t[cb], in0=bp2, in1=bpp_t[cb])

        # --- K = Wk'^T x  (no bias needed: softmax-invariant) ---
        for cb in range(CT):
            for jc in range(8):
                kp = pps.tile([128, 512], F32, name=f"{r}kp{cb}_{jc}", tag="mm")
                for t in range(CT):
                    nc.tensor.matmul(kp, wkb[t][:, cb * 128:(cb + 1) * 128],
                                     xb_t[t][:, jc * 512:(jc + 1) * 512],
                                     start=(t == 0), stop=(t == CT - 1))
                _copy([nc.scalar, nc.vector][jc % 2], nc,
                      k_sb[cb][:, jc * 512:(jc + 1) * 512], kp)

        # --- Q = Wq'^T x_q + bq_dyn ---
        for cb in range(CT):
            for ic in range(ICN):
                qp = pps.tile([128, 512], F32, name=f"{r}qp{cb}_{ic}", tag="mm")
                for t in range(CT):
                    nc.tensor.matmul(
                        qp, wqb[t][:, cb * 128:(cb + 1) * 128],
                        xb_t[t][:, ic * ICW:(ic + 1) * ICW],
                        start=(t == 0), stop=(t == CT - 1))
                nc.scalar.activation(out=q_sb[cb][:, ic * ICW:(ic + 1) * ICW],
                                     in_=qp, func=AF.Identity, bias=bqd_t[cb],
                                     scale=1.0)

        # --- VPT = x^T Wvp' ---
        for jb in range(JBN):
            vp = pps.tile([128, 512], F32, name=f"{r}vp{jb}", tag="mm")
            for t in range(CT):
                nc.tensor.matmul(vp, xb_t[t][:, jb * 128:(jb + 1) * 128],
                                 wvpb[t], start=(t == 0), stop=(t == CT - 1))
            _copy([nc.scalar, nc.vector][jb % 2], nc, vt_sb[jb], vp)

    wb.release()
    xbpool.release()
    gps.release()

    # ============ PHASE 2: attention ============
    ep = ctx.enter_context(tc.tile_pool(name=r + "ep", bufs=4))
    denp = ctx.enter_context(tc.tile_pool(name=r + "den", bufs=2))
    fin = ctx.enter_context(tc.tile_pool(name=r + "fin", bufs=2))
    ops = ctx.enter_context(tc.tile_pool(name=r + "ops", bufs=1, space="PSUM"))

    for ic in range(ICN):
        # residual x reload for this chunk (overlaps attention)
        xq_pre = []
        for cb in range(CT):
            xp = fin.tile([128, ICW], F32, name=f"{r}xq{cb}_{ic}", tag=f"xq{cb}",
                          bufs=1)
            nc.sync.dma_start(
                out=xp,
                in_=XF[cb * 128:(cb + 1) * 128, ic * ICW:(ic + 1) * ICW])
            xq_pre.append(xp)

        o_ps = [ops.tile([128, ICW], F32, name=f"{r}o{cb}_{ic}", tag=f"o{cb}")
                for cb in range(CT)]
        den_v = denp.tile([128, ICW], F32, name=f"{r}dv{ic}", tag="dv")
        den_g = denp.tile([128, ICW], F32, name=f"{r}dg{ic}", tag="dg")

        e_tiles = {}

        def s_block(jb):
            st = pps.tile([128, ICW], F32, name=f"{r}s{ic}_{jb}", tag="mm")
            for t in range(CT):
                nc.tensor.matmul(
                    st, k_sb[t][:, jb * 128:(jb + 1) * 128],
                    q_sb[t][:, ic * ICW:(ic + 1) * ICW],
                    start=(t == 0), stop=(t == CT - 1))
            e = ep.tile([128, ICW], BF16, name=f"{r}e{ic}_{jb}", tag="e")
            nc.scalar.activation(out=e, in_=st, func=AF.Exp, scale=INV)
            if jb % 2 == 0:
                if jb == 0:
                    nc.vector.tensor_copy(out=den_v, in_=e)
                else:
                    nc.vector.tensor_add(out=den_v, in0=den_v, in1=e)
            else:
                if jb == 1:
                    nc.gpsimd.tensor_copy(out=den_g, in_=e)
                else:
                    nc.gpsimd.tensor_add(out=den_g, in0=den_g, in1=e)
            e_tiles[jb] = e

        def o_block(jb):
            e = e_tiles.pop(jb)
            for cb in range(CT):
                nc.tensor.matmul(o_ps[cb], vt_sb[jb][:, cb * 128:(cb + 1) * 128],
                                 e, start=(jb == 0), stop=(jb == JBN - 1))

        s_block(0)
        for jb in range(1, JBN):
            s_block(jb)
            o_block(jb - 1)
        o_block(JBN - 1)

        # ---- finalize chunk ----
        dt = denp.tile([128, ICW], F32, name=f"{r}dt{ic}", tag="dt")
        nc.vector.tensor_add(out=dt, in0=den_v, in1=den_g)
        dps = pps.tile([128, ICW], F32, name=f"{r}dp{ic}", tag="mm")
        nc.tensor.matmul(dps, ones_t, dt, start=True, stop=True)
        rb = fin.tile([128, ICW], F32, name=f"{r}rb{ic}", tag="rb")
        nc.vector.reciprocal(out=rb, in_=dps)
        for cb in range(CT):
            t1 = fin.tile([128, ICW], F32, name=f"{r}t1{cb}_{ic}", tag="t1")
            nc.vector.tensor_mul(out=t1, in0=o_ps[cb], in1=rb)
            t2 = fin.tile([128, ICW], F32, name=f"{r}t2{cb}_{ic}", tag="t2")
            nc.scalar.activation(out=t2, in_=t1, func=AF.Identity,
                                 bias=bppd_t[cb], scale=1.0)
            ot = fin.tile([128, ICW], F32, name=f"{r}ot{cb}_{ic}", tag="ot")
            geng = nc.gpsimd if cb % 2 else nc.vector
            geng.tensor_add(out=ot, in0=t2, in1=xq_pre[cb])
            nc.sync.dma_start(
                out=OUT[cb * 128:(cb + 1) * 128, ic * ICW:(ic + 1) * ICW],
                in_=ot)


def _build(reps=1):
    from contextlib import ExitStack as ES
    nc = bacc.Bacc()
    tens = {
        "XF": nc.dram_tensor("XF", [C, N], F32, kind="ExternalInput"),
        "WQT": nc.dram_tensor("WQT", [CT, 128, C], F32, kind="ExternalInput"),
        "WKT": nc.dram_tensor("WKT", [CT, 128, C], F32, kind="ExternalInput"),
        "WVPT": nc.dram_tensor("WVPT", [CT, 128, C], F32, kind="ExternalInput"),
        "CVEC": nc.dram_tensor("CVEC", [128, 16], F32, kind="ExternalInput"),
        "GM": nc.dram_tensor("GM", [128, 128], F32, kind="ExternalInput"),
        "OUT": nc.dram_tensor("OUT", [C, NQ], F32, kind="ExternalOutput"),
    }
    with tile.TileContext(nc) as tc:
        for rep in range(reps):
            with ES() as ctx:
                _emit(nc, tc, ctx, tens, rep)
    nc.finalize()
    return nc


_NC_CACHE = {}


def _get_nc(reps=1):
    if reps not in _NC_CACHE:
        _NC_CACHE[reps] = _build(reps)
    return _NC_CACHE[reps]


def _prep_inputs(x, gn_scale, gn_bias, wq, bq, wk, bk, wv, bv, wp, bp):
    x = np.ascontiguousarray(np.asarray(x, dtype=np.float32))
    B = x.shape[0]
    xb = x.reshape(B, C, N)
    f32 = lambda v: np.ascontiguousarray(np.asarray(v, dtype=np.float32))
    wq, wk, wv, wp = f32(wq), f32(wk), f32(wv), f32(wp)
    bq, bv, bp = f32(bq), f32(bv), f32(bp)
    wvp = wp @ wv  # VP = (Wp Wv) @ h
    common = {
        "WQT": f32(wq.T.reshape(CT, 128, C)),
        "WKT": f32(wk.T.reshape(CT, 128, C)),
        "WVPT": f32(wvp.T.reshape(CT, 128, C)),
        "CVEC": np.ascontiguousarray(np.concatenate(
            [v.reshape(CT, 128).T for v in
             [bq, (wp @ bv + bp).astype(np.float32),
              f32(gn_scale), f32(gn_bias)]], axis=1), dtype=np.float32),
        "GM": np.kron(np.eye(8, dtype=np.float32),
                      np.full((16, 16), 1.0 / 16.0, np.float32)),
    }
    in_maps = []
    for core in range(8):
        b, h = core // 2, core % 2
        m = dict(common)
        if h == 0:
            m["XF"] = xb[b]
        else:
            m["XF"] = np.ascontiguousarray(
                np.concatenate([xb[b][:, NQ:], xb[b][:, :NQ]], axis=1))
        in_maps.append(m)
    return in_maps, B


def kernel(**inputs):
    nc = _get_nc(1)
    in_maps, B = _prep_inputs(**inputs)
    res = run_bass_kernel_spmd(nc, in_maps, core_ids=list(range(8)))
    out = np.empty((B, C, N), dtype=np.float32)
    for core in range(8):
        b, h = core // 2, core % 2
        out[b][:, h * NQ:(h + 1) * NQ] = res.results[core]["OUT"]
    return out.reshape(B, C, 64, 64)


# revision 34
# speedup vs baseline: 1.0873x; 1.0873x over previous
"""Trainium2 Bass kernel for AttnBlock (GroupNorm + single-head spatial
self-attention + projection + residual).

Sharding: 8 cores = 4 batches x 2 query-halves. Each core computes
GN-folded K / VP for its batch (duplicated within the pair) and
attention for its half of the 4096 query positions. No collectives.
The host rotates each core's XF so its 2048 query columns are always
columns 0:NQ (attention is permutation-invariant over keys).

Math per core (batch b, N=4096 keys, NQ=2048 queries):
  GN: h = a*x + c2 per channel (a = gns/std, c2 = gnb - mean*a)
  GN is folded into the weights (W' = diag(a) W, in-place scale) and
  biases (via c2p = c2/a so scaled weights can be reused):
  K   = Wk'^T x                     [C, N]  (K-side affine offset is
                                    per-query-constant -> softmax-invariant,
                                    dropped entirely; no bias on K)
  Q   = Wq'^T x + (Wq'^T c2p + bq)  [C, NQ]
  VPT = x^T Wvp' (+ c2p^T Wvp' folded into bpp) with Wvp = (Wp Wv).T
        so attention output is already projected (no separate proj pass).
  S^T = K^T Q * C^-0.5 -> E = exp(S^T)  [N, NQ] (no max-sub; scores O(5))
  O   = VPT^T @ E (unnorm.)         [C, NQ]; den = sum_j E (ones-matmul
                                    for the cross-partition reduce)
  out = x_q + O/den + bpp_dyn       bpp_dyn = Wp@bv + bp + Wvp'^T c2p
Phase 1 runs f32r (x resident, no converts); K/Q/VPT outputs and the
attention matmuls are bf16 (FWL fast weight loads, half SBUF).
"""
import math
import numpy as np

import concourse.bass as bass
import concourse.bacc as bacc
import concourse.tile as tile
from concourse import mybir
from concourse.bass_utils import run_bass_kernel_spmd

F32 = mybir.dt.float32
F32R = mybir.dt.float32r
BF16 = mybir.dt.bfloat16
AF = mybir.ActivationFunctionType
ALU = mybir.AluOpType

C = 512          # channels
N = 4096         # spatial positions (keys)
NQ = 2048        # queries per core
CT = 4           # channel tiles of 128
ICN = 4          # query chunks per core
ICW = 512        # query chunk width
JBN = 32         # j-blocks (128 wide)
GROUPS = 32
EPS = 1e-6
INV = 1.0 / math.sqrt(C)
BN_FMAX = 512


def _copy(eng, nc, out, in_):
    if eng is nc.scalar:
        nc.scalar.copy(out=out, in_=in_)
    else:
        eng.tensor_copy(out=out, in_=in_)


def _emit(nc, tc, ctx, tens, rep):
    r = f"r{rep}_"
    XF = tens["XF"]
    WQT, WKT, WVPT = tens["WQT"], tens["WKT"], tens["WVPT"]
    GM, OUT = tens["GM"], tens["OUT"]

    const = ctx.enter_context(tc.tile_pool(name=r + "const", bufs=1))
    kqpool = ctx.enter_context(tc.tile_pool(name=r + "kq", bufs=1))
    vpool = ctx.enter_context(tc.tile_pool(name=r + "vt", bufs=1))
    xpool = ctx.enter_context(tc.tile_pool(name=r + "xp", bufs=1))
    pps = ctx.enter_context(tc.tile_pool(name=r + "pps", bufs=4, space="PSUM"))
    # transient pools (LIFO: released in reverse creation order)
    wstage = tc.alloc_tile_pool(name=r + "wstage", bufs=1)
    pgn = tc.alloc_tile_pool(name=r + "pgn", bufs=2)
    gps = tc.alloc_tile_pool(name=r + "gps", bufs=2, space="PSUM")

    # ---------------- constants + weights (scalar queue, parallel DMA) ----
    gm_t = const.tile([128, 128], F32, name=r + "gm")
    nc.scalar.dma_start(out=gm_t, in_=GM[:, :])
    cvec = const.tile([128, 16], F32, name=r + "cvec")
    nc.scalar.dma_start(out=cvec, in_=tens["CVEC"][:, :])
    bq_t = [cvec[:, cb:cb + 1] for cb in range(CT)]
    bpp_t = [cvec[:, 4 + cb:5 + cb] for cb in range(CT)]
    gns_t = [cvec[:, 8 + t:9 + t] for t in range(CT)]
    gnb_t = [cvec[:, 12 + t:13 + t] for t in range(CT)]
    eps_t = const.tile([128, 1], F32, name=r + "eps")
    nc.vector.memset(eps_t, EPS)
    ones_t = const.tile([128, 128], F32, name=r + "ones")
    nc.vector.memset(ones_t, 1.0)
    a_t = [const.tile([128, 1], F32, name=f"{r}a{t}", tag=f"a{t}") for t in range(CT)]
    c2p_t = [const.tile([128, 2], F32R, name=f"{r}c2p{t}", tag=f"c2p{t}")
             for t in range(CT)]
    bqd_t = [const.tile([128, 1], F32, name=f"{r}bqd{cb}", tag=f"bqd{cb}")
             for cb in range(CT)]
    bppd_t = [const.tile([128, 1], F32, name=f"{r}bppd{cb}", tag=f"bppd{cb}")
              for cb in range(CT)]

    wq_w = [wstage.tile([128, C], F32R, name=f"{r}wq{t}", tag=f"wq{t}")
            for t in range(CT)]
    wk_w = [wstage.tile([128, C], F32R, name=f"{r}wk{t}", tag=f"wk{t}")
            for t in range(CT)]
    wvp_w = [wstage.tile([128, C], F32R, name=f"{r}wvp{t}", tag=f"wvp{t}")
             for t in range(CT)]
    for t in range(CT):
        nc.scalar.dma_start(out=wk_w[t], in_=WKT[t].bitcast(F32R))
    for t in range(CT):
        nc.scalar.dma_start(out=wq_w[t], in_=WQT[t].bitcast(F32R))
    for t in range(CT):
        nc.scalar.dma_start(out=wvp_w[t], in_=WVPT[t].bitcast(F32R))

    # ---------------- x load (sync queue) + per-tile stats chain ----------
    x_t = [xpool.tile([128, N], F32R, name=f"{r}x{t}", tag=f"x{t}")
           for t in range(CT)]
    for t in range(CT):
        stats = pgn.tile([128, N // BN_FMAX, 6], F32, name=f"{r}st{t}", tag="st")
        for s in range(N // BN_FMAX):
            sl = slice(s * BN_FMAX, (s + 1) * BN_FMAX)
            nc.sync.dma_start(out=x_t[t][:, sl],
                              in_=XF[t * 128:(t + 1) * 128, sl].bitcast(F32R))
            nc.vector.bn_stats(out=stats[:, s, :], in_=x_t[t][:, sl].bitcast(F32))
        # ---- group stats for tile t -> a_t, c2p_t; scale weights in place
        mv = pgn.tile([128, 2], F32, name=f"{r}mv{t}", tag="mv")
        nc.vector.bn_aggr(out=mv, in_=stats)
        t2 = pgn.tile([128, 2], F32, name=f"{r}t2{t}", tag="t2")
        nc.vector.tensor_copy(out=t2[:, 0:1], in_=mv[:, 0:1])
        sq = pgn.tile([128, 1], F32, name=f"{r}sq{t}", tag="sq")
        nc.vector.tensor_mul(out=sq, in0=mv[:, 0:1], in1=mv[:, 0:1])
        nc.vector.tensor_add(out=t2[:, 1:2], in0=mv[:, 1:2], in1=sq)
        chp = gps.tile([128, 2], F32, name=f"{r}chp{t}", tag="gp")
        nc.tensor.matmul(chp, gm_t, t2, start=True, stop=True)
        ch = pgn.tile([128, 2], F32, name=f"{r}ch{t}", tag="ch")
        nc.vector.tensor_copy(out=ch, in_=chp)
        gmean, gmsq = ch[:, 0:1], ch[:, 1:2]
        sg = pgn.tile([128, 1], F32, name=f"{r}sg{t}", tag="sg")
        nc.vector.tensor_mul(out=sg, in0=gmean, in1=gmean)
        gv = pgn.tile([128, 1], F32, name=f"{r}gv{t}", tag="gv")
        nc.vector.tensor_sub(out=gv, in0=gmsq, in1=sg)
        nc.scalar.activation(out=gv, in_=gv, func=AF.Sqrt, bias=eps_t, scale=1.0)
        nc.vector.reciprocal(out=gv, in_=gv)
        nc.vector.tensor_mul(out=a_t[t], in0=gv, in1=gns_t[t])
        # c2p = c2/a = gnb/a - gmean  (final write rounds to f32r)
        ia = pgn.tile([128, 1], F32, name=f"{r}ia{t}", tag="ia")
        nc.vector.reciprocal(out=ia, in_=a_t[t])
        c2f = pgn.tile([128, 1], F32, name=f"{r}c2f{t}", tag="c2f")
        nc.vector.tensor_mul(out=c2f, in0=gnb_t[t], in1=ia)
        nc.vector.tensor_sub(out=c2p_t[t][:, 0:1], in0=c2f, in1=gmean)
        nc.vector.tensor_sub(out=c2p_t[t][:, 1:2], in0=c2f, in1=gmean)
        # in-place scale: w' = a * w   (scalar.activation is the fast path)
        nc.scalar.activation(out=wk_w[t], in_=wk_w[t].bitcast(F32),
                             func=AF.Identity, scale=a_t[t])
        nc.scalar.activation(out=wq_w[t], in_=wq_w[t].bitcast(F32),
                             func=AF.Identity, scale=a_t[t])
        nc.scalar.activation(out=wvp_w[t], in_=wvp_w[t].bitcast(F32),
                             func=AF.Identity, scale=a_t[t])

    pgn.release()

    # resident K, Q (bf16)
    k_sb = [kqpool.tile([128, N], BF16, name=f"{r}k{t}", tag=f"k{t}")
            for t in range(CT)]
    q_sb = [kqpool.tile([128, NQ], BF16, name=f"{r}q{t}", tag=f"q{t}")
            for t in range(CT)]
    vt_sb = [vpool.tile([128, 512], BF16, name=f"{r}vt{jb}", tag=f"vt{jb}")
             for jb in range(JBN)]

    # ---- K = Wk'^T x ----
    for cb in range(CT):
        for jc in range(8):
            kp = pps.tile([128, 512], F32, name=f"{r}kp{cb}_{jc}", tag="mm")
            for t in range(CT):
                nc.tensor.matmul(kp, wk_w[t][:, cb * 128:(cb + 1) * 128],
                                 x_t[t][:, jc * 512:(jc + 1) * 512],
                                 start=(t == 0), stop=(t == CT - 1))
            _copy([nc.scalar, nc.vector][jc % 2], nc,
                  k_sb[cb][:, jc * 512:(jc + 1) * 512], kp)

    # ---- dynamic biases (run while K copies drain):
    #      bq_dyn = bq + Wq'^T c2p ; bpp_dyn = bpp + Wvp'^T c2p
    for cb in range(CT):
        bp1 = gps.tile([128, 2], F32, name=f"{r}bp1{cb}", tag="gp")
        for t in range(CT):
            nc.tensor.matmul(bp1, wq_w[t][:, cb * 128:(cb + 1) * 128],
                             c2p_t[t], start=(t == 0), stop=(t == CT - 1))
        nc.vector.tensor_add(out=bqd_t[cb], in0=bp1[:, 0:1], in1=bq_t[cb])
        bp2 = gps.tile([128, 2], F32, name=f"{r}bp2{cb}", tag="gp")
        for t in range(CT):
            nc.tensor.matmul(bp2, wvp_w[t][:, cb * 128:(cb + 1) * 128],
                             c2p_t[t], start=(t == 0), stop=(t == CT - 1))
        nc.vector.tensor_add(out=bppd_t[cb], in0=bp2[:, 0:1], in1=bpp_t[cb])

    # ---- Q = Wq'^T x_q + bq_dyn ----
    for cb in range(CT):
        for ic in range(ICN):
            qp = pps.tile([128, 512], F32, name=f"{r}qp{cb}_{ic}", tag="mm")
            for t in range(CT):
                nc.tensor.matmul(qp, wq_w[t][:, cb * 128:(cb + 1) * 128],
                                 x_t[t][:, ic * ICW:(ic + 1) * ICW],
                                 start=(t == 0), stop=(t == CT - 1))
            nc.scalar.activation(out=q_sb[cb][:, ic * ICW:(ic + 1) * ICW],
                                 in_=qp, func=AF.Identity, bias=bqd_t[cb],
                                 scale=1.0)

    # ---- VPT = x^T Wvp' ----
    for jb in range(JBN):
        vp = pps.tile([128, 512], F32, name=f"{r}vp{jb}", tag="mm")
        for t in range(CT):
            nc.tensor.matmul(vp, x_t[t][:, jb * 128:(jb + 1) * 128],
                             wvp_w[t], start=(t == 0), stop=(t == CT - 1))
        _copy([nc.scalar, nc.vector][jb % 2], nc, vt_sb[jb], vp)

    wstage.release()
    gps.release()

    # ============ PHASE 2: attention ============
    ep = ctx.enter_context(tc.tile_pool(name=r + "ep", bufs=4))
    denp = ctx.enter_context(tc.tile_pool(name=r + "den", bufs=2))
    fin = ctx.enter_context(tc.tile_pool(name=r + "fin", bufs=2))
    ops = ctx.enter_context(tc.tile_pool(name=r + "ops", bufs=1, space="PSUM"))

    for ic in range(ICN):
        o_ps = [ops.tile([128, ICW], F32, name=f"{r}o{cb}_{ic}", tag=f"o{cb}")
                for cb in range(CT)]
        den_v = denp.tile([128, ICW], F32, name=f"{r}dv{ic}", tag="dv")
        den_g = denp.tile([128, ICW], F32, name=f"{r}dg{ic}", tag="dg")

        e_tiles = {}

        def s_block(jb, ic=ic, den_v=den_v, den_g=den_g, e_tiles=e_tiles):
            st = pps.tile([128, ICW], F32, name=f"{r}s{ic}_{jb}", tag="mm")
            for t in range(CT):
                nc.tensor.matmul(
                    st, k_sb[t][:, jb * 128:(jb + 1) * 128],
                    q_sb[t][:, ic * ICW:(ic + 1) * ICW],
                    start=(t == 0), stop=(t == CT - 1))
            e = ep.tile([128, ICW], BF16, name=f"{r}e{ic}_{jb}", tag="e")
            nc.scalar.activation(out=e, in_=st, func=AF.Exp, scale=INV)
            if jb % 2 == 0:
                if jb == 0:
                    nc.vector.tensor_copy(out=den_v, in_=e)
                else:
                    nc.vector.tensor_add(out=den_v, in0=den_v, in1=e)
            else:
                if jb == 1:
                    nc.gpsimd.tensor_copy(out=den_g, in_=e)
                else:
                    nc.gpsimd.tensor_add(out=den_g, in0=den_g, in1=e)
            e_tiles[jb] = e

        def o_block(jb, o_ps=o_ps, e_tiles=e_tiles):
            e = e_tiles.pop(jb)
            for cb in range(CT):
                nc.tensor.matmul(o_ps[cb], vt_sb[jb][:, cb * 128:(cb + 1) * 128],
                                 e, start=(jb == 0), stop=(jb == JBN - 1))

        s_block(0)
        for jb in range(1, JBN):
            s_block(jb)
            o_block(jb - 1)
        o_block(JBN - 1)

        # ---- finalize chunk: den reduce via ones-matmul, normalize, residual
        dps = pps.tile([128, ICW], F32, name=f"{r}dp{ic}", tag="mm")
        nc.tensor.matmul(dps, ones_t, den_v, start=True, stop=False)
        nc.tensor.matmul(dps, ones_t, den_g, start=False, stop=True)
        rb = fin.tile([128, ICW], F32, name=f"{r}rb{ic}", tag="rb")
        nc.vector.reciprocal(out=rb, in_=dps)
        for cb in range(CT):
            t1 = fin.tile([128, ICW], F32, name=f"{r}t1{cb}_{ic}", tag="t1")
            nc.vector.tensor_mul(out=t1, in0=o_ps[cb], in1=rb)
            t2 = fin.tile([128, ICW], F32, name=f"{r}t2{cb}_{ic}", tag="t2")
            nc.scalar.activation(out=t2, in_=t1, func=AF.Identity,
                                 bias=bppd_t[cb], scale=1.0)
            ot = fin.tile([128, ICW], F32, name=f"{r}ot{cb}_{ic}", tag="ot")
            geng = nc.gpsimd if cb % 2 else nc.vector
            geng.tensor_add(out=ot, in0=t2,
                            in1=x_t[cb][:, ic * ICW:(ic + 1) * ICW].bitcast(F32))
            nc.sync.dma_start(
                out=OUT[cb * 128:(cb + 1) * 128, ic * ICW:(ic + 1) * ICW],
                in_=ot)


def _build(reps=1):
    from contextlib import ExitStack as ES
    nc = bacc.Bacc()
    tens = {
        "XF": nc.dram_tensor("XF", [C, N], F32, kind="ExternalInput"),
        "WQT": nc.dram_tensor("WQT", [CT, 128, C], F32, kind="ExternalInput"),
        "WKT": nc.dram_tensor("WKT", [CT, 128, C], F32, kind="ExternalInput"),
        "WVPT": nc.dram_tensor("WVPT", [CT, 128, C], F32, kind="ExternalInput"),
        "CVEC": nc.dram_tensor("CVEC", [128, 16], F32, kind="ExternalInput"),
        "GM": nc.dram_tensor("GM", [128, 128], F32, kind="ExternalInput"),
        "OUT": nc.dram_tensor("OUT", [C, NQ], F32, kind="ExternalOutput"),
    }
    with tile.TileContext(nc) as tc:
        for rep in range(reps):
            with ES() as ctx:
                _emit(nc, tc, ctx, tens, rep)
    nc.finalize()
    return nc


_NC_CACHE = {}


def _get_nc(reps=1):
    if reps not in _NC_CACHE:
        _NC_CACHE[reps] = _build(reps)
    return _NC_CACHE[reps]


def _prep_inputs(x, gn_scale, gn_bias, wq, bq, wk, bk, wv, bv, wp, bp):
    x = np.ascontiguousarray(np.asarray(x, dtype=np.float32))
    B = x.shape[0]
    xb = x.reshape(B, C, N)
    f32 = lambda v: np.ascontiguousarray(np.asarray(v, dtype=np.float32))
    wq, wk, wv, wp = f32(wq), f32(wk), f32(wv), f32(wp)
    bq, bv, bp = f32(bq), f32(bv), f32(bp)
    wvp = wp @ wv  # VP = (Wp Wv) @ h
    common = {
        "WQT": f32(wq.T.reshape(CT, 128, C)),
        "WKT": f32(wk.T.reshape(CT, 128, C)),
        "WVPT": f32(wvp.T.reshape(CT, 128, C)),
        "CVEC": np.ascontiguousarray(np.concatenate(
            [v.reshape(CT, 128).T for v in
             [bq, (wp @ bv + bp).astype(np.float32),
              f32(gn_scale), f32(gn_bias)]], axis=1), dtype=np.float32),
        "GM": np.kron(np.eye(8, dtype=np.float32),
                      np.full((16, 16), 1.0 / 16.0, np.float32)),
    }
    in_maps = []
    for core in range(8):
        b, h = core // 2, core % 2
        m = dict(common)
        if h == 0:
            m["XF"] = xb[b]
        else:
            m["XF"] = np.ascontiguousarray(
                np.concatenate([xb[b][:, NQ:], xb[b][:, :NQ]], axis=1))
        in_maps.append(m)
    return in_maps, B


def kernel(**inputs):
    nc = _get_nc(1)
    in_maps, B = _prep_inputs(**inputs)
    res = run_bass_kernel_spmd(nc, in_maps, core_ids=list(range(8)))
    out = np.empty((B, C, N), dtype=np.float32)
    for core in range(8):
        b, h = core // 2, core % 2
        out[b][:, h * NQ:(h + 1) * NQ] = res.results[core]["OUT"]
    return out.reshape(B, C, 64, 64)
